# revision 9
# baseline (speedup 1.0000x reference)
"""Trainium2 Bass kernel for nn_CrossAttentionBlock (GroupNorm + 1x1-conv Q +
cross-attention over cond + output projection + residual).

Full-input contract: kernel(**inputs) takes the complete unsharded inputs and
returns the full [16, 512, 64, 64] float32 output.  Internally shards
data-parallel over batch across 8 NeuronCores (2 batches per core), runs one
SPMD Bass/Tile kernel via run_bass_kernel_spmd, and concatenates the results.

Layout strategy (per core, per batch, channels-first [C, HW] everywhere):
  x [512, 4096] -> groupnorm stats (DVE sum + ACT square/accum, tiny PE
  indicator matmuls for the 16-channel group reduce/scatter) -> per-channel
  scale/bias -> per-hw-chunk (512 cols):
    xn = ACT(x*sc + tc) in bf16
    q  = qwT.T @ xn (+q_b fused into the PSUM->SBUF copy)   [C, 512]  PE
    per head h (hd=64): logits^T = kT_h.T @ q_h   [77, 512]  PE
      (heads of a pair sit at partition bases 0/64 -> concurrent PE
       row-groups for the K=64 QK matmuls)
    exp via ACT -> per-head [77, 512] bf16 tiles (no max subtraction;
      logits are O(10) so fp32 exp is safe)
    sums_h = ones.T @ exp_h  (M=1 matmuls into PSUM partitions
      {0,32,64,96} of two banks -> 4-way PE col-group concurrency)
    recip = DVE reciprocal; GpSimd partition_broadcast recip row over the
      64 output channels of each head
    av = v_h @ exp_h [64, 512] PE, two heads pair-packed at bases {0,64};
      normalize+bf16-cast fused into the PSUM->SBUF copy (DVE tensor_mul)
    out = pwT.T @ av_norm (+proj_b via K=1 matmul) + x  (DVE add) -> DMA
Weights are transposed/cast to bf16 on the host (layout prep only).
"""

import sys

for _p in ("/opt/trn_rl_repo",):
    if _p not in sys.path:
        sys.path.append(_p)

from contextlib import ExitStack

import numpy as np
import ml_dtypes

import concourse.bacc as bacc
import concourse.tile as tile
from concourse import mybir
from concourse.bass_utils import run_bass_kernel_spmd

BF16 = ml_dtypes.bfloat16

N_CORES = 8
B, C, H, W = 16, 512, 64, 64
HW = H * W                      # 4096
L, CD = 77, 768
NH, HD = 8, 64                  # heads, head dim
NG, GS = 32, 16                 # groups, channels per group
EPS = 1e-6
B_LOC = B // N_CORES            # 2
NT = C // 128                   # 4 channel tiles
KT = CD // 128                  # 6 cond-dim tiles
CH = 512                        # hw chunk
NCH = HW // CH                  # 8
GPT = 128 // GS                 # 8 groups per 128-channel tile
GN_N = float(GS * HW)           # elements per group


def _build_nc():
    f32 = mybir.dt.float32
    bf16 = mybir.dt.bfloat16
    nc = bacc.Bacc("TRN2", target_bir_lowering=False, debug=False)

    x_d = nc.dram_tensor("x", [B_LOC, C, HW], f32, kind="ExternalInput").ap()
    condT_d = nc.dram_tensor("condT", [B_LOC, CD, L], bf16, kind="ExternalInput").ap()
    qwT_d = nc.dram_tensor("qwT", [C, C], bf16, kind="ExternalInput").ap()
    kwT_d = nc.dram_tensor("kwT", [CD, C], bf16, kind="ExternalInput").ap()
    vwT_d = nc.dram_tensor("vwT", [CD, C], bf16, kind="ExternalInput").ap()
    pwT_d = nc.dram_tensor("pwT", [C, C], bf16, kind="ExternalInput").ap()
    gamma_d = nc.dram_tensor("gamma", [C, 1], f32, kind="ExternalInput").ap()
    beta_d = nc.dram_tensor("beta", [C, 1], f32, kind="ExternalInput").ap()
    qb_d = nc.dram_tensor("qb", [C, 1], f32, kind="ExternalInput").ap()
    kb_d = nc.dram_tensor("kb", [C, 1], f32, kind="ExternalInput").ap()
    vb_d = nc.dram_tensor("vb", [1, C], f32, kind="ExternalInput").ap()
    pb_d = nc.dram_tensor("pb", [1, C], bf16, kind="ExternalInput").ap()
    scale_d = nc.dram_tensor("scale", [1, 1], f32, kind="ExternalInput").ap()
    g16_d = nc.dram_tensor("g16", [128, GPT], f32, kind="ExternalInput").ap()
    g16T_d = nc.dram_tensor("g16T", [GPT, 128], f32, kind="ExternalInput").ap()
    out_d = nc.dram_tensor("out", [B_LOC, C, HW], f32, kind="ExternalOutput").ap()

    AO = mybir.AluOpType
    AF = mybir.ActivationFunctionType

    with tile.TileContext(nc) as tc, ExitStack() as ctx:
        # --- pools ---
        wp = ctx.enter_context(tc.tile_pool(name="weights", bufs=1))
        sb1 = ctx.enter_context(tc.tile_pool(name="work1", bufs=1))
        sb2 = ctx.enter_context(tc.tile_pool(name="work2", bufs=2))
        sb3 = ctx.enter_context(tc.tile_pool(name="work3", bufs=3))
        # PSUM: q(2) + at(2, shared qk/av) + sums(2x1) + o(2) = 8 banks
        ps_q = ctx.enter_context(tc.tile_pool(name="ps_q", bufs=2, space="PSUM"))
        ps_at = ctx.enter_context(tc.tile_pool(name="ps_at", bufs=2, space="PSUM"))
        ps_sm = ctx.enter_context(tc.tile_pool(name="ps_sm", bufs=2, space="PSUM"))
        ps_o = ctx.enter_context(tc.tile_pool(name="ps_o", bufs=2, space="PSUM"))

        # --- load persistent weights/constants ---
        qwT = [wp.tile([128, C], bf16, tag=f"qwT{j}", name=f"qwT{j}")
               for j in range(NT)]
        pwT = [wp.tile([128, C], bf16, tag=f"pwT{j}", name=f"pwT{j}")
               for j in range(NT)]
        kwT = [wp.tile([128, C], bf16, tag=f"kwT{j}", name=f"kwT{j}")
               for j in range(KT)]
        vwT = [wp.tile([128, C], bf16, tag=f"vwT{j}", name=f"vwT{j}")
               for j in range(KT)]
        for j in range(NT):
            nc.sync.dma_start(qwT[j][:], qwT_d[128 * j:128 * (j + 1), :])
            nc.sync.dma_start(pwT[j][:], pwT_d[128 * j:128 * (j + 1), :])
        for j in range(KT):
            nc.sync.dma_start(kwT[j][:], kwT_d[128 * j:128 * (j + 1), :])
            nc.sync.dma_start(vwT[j][:], vwT_d[128 * j:128 * (j + 1), :])

        g16 = wp.tile([128, GPT], f32, tag="g16")
        nc.sync.dma_start(g16[:], g16_d[:, :])
        g16T = wp.tile([GPT, 128], f32, tag="g16T")
        nc.sync.dma_start(g16T[:], g16T_d[:, :])

        # column vectors: 0-3 gamma, 4-7 beta, 8-11 qb, 12-15 kb (per c-tile)
        colv = wp.tile([128, 16], f32, tag="colv")
        for t in range(NT):
            s = slice(128 * t, 128 * (t + 1))
            nc.sync.dma_start(colv[:, t:t + 1], gamma_d[s, :])
            nc.sync.dma_start(colv[:, 4 + t:5 + t], beta_d[s, :])
            nc.sync.dma_start(colv[:, 8 + t:9 + t], qb_d[s, :])
            nc.sync.dma_start(colv[:, 12 + t:13 + t], kb_d[s, :])
        vb_row = wp.tile([1, C], f32, tag="vb_row")
        nc.sync.dma_start(vb_row[:], vb_d[:, :])
        pb_row = wp.tile([1, C], bf16, tag="pb_row")
        nc.sync.dma_start(pb_row[:], pb_d[:, :])
        s11 = wp.tile([1, 1], f32, tag="s11")
        nc.sync.dma_start(s11[:], scale_d[:, :])
        scale_col = wp.tile([128, 1], f32, tag="scale_col")
        nc.gpsimd.partition_broadcast(scale_col[:], s11[:])
        ones_bf = wp.tile([1, CH], bf16, tag="ones_bf")
        nc.gpsimd.memset(ones_bf[:], 1.0)
        ones77 = wp.tile([L, 64], bf16, tag="ones77")
        nc.gpsimd.memset(ones77[:], 1.0)
        # k bias pre-scaled by `scale` (folded into the kT copy)
        kbs = wp.tile([128, NT], f32, tag="kbs")
        for t in range(NT):
            nc.vector.tensor_mul(kbs[:, t:t + 1], colv[:, 12 + t:13 + t],
                                 scale_col[:])
        # v bias broadcast over the 77 cond rows (batch independent)
        vb_bc = wp.tile([L, C], f32, tag="vb_bc")
        nc.gpsimd.partition_broadcast(vb_bc[:], vb_row[:])

        for b in range(B_LOC):
            # ---------- load x ----------
            xb = [sb1.tile([128, HW], f32, tag=f"x{t}", name=f"x{t}")
                  for t in range(NT)]
            for t in range(NT):
                nc.sync.dma_start(xb[t][:], x_d[b, 128 * t:128 * (t + 1), :])

            # ---------- groupnorm stats ----------
            # stats cols: 2t = sum(x), 2t+1 = sum(x^2) per channel tile
            stats = sb2.tile([128, 2 * NT], f32, tag="stats")
            s2c = sb2.tile([128, NCH], f32, tag="s2c")
            trash = sb2.tile([128, CH], f32, tag="trash")
            for t in range(NT):
                nc.vector.tensor_reduce(stats[:, 2 * t:2 * t + 1], xb[t][:],
                                        axis=mybir.AxisListType.X, op=AO.add)
                for cix in range(NCH):
                    nc.scalar.activation(trash[:], xb[t][:, CH * cix:CH * (cix + 1)],
                                         AF.Square,
                                         accum_out=s2c[:, cix:cix + 1])
                nc.vector.tensor_reduce(stats[:, 2 * t + 1:2 * t + 2], s2c[:],
                                        axis=mybir.AxisListType.X, op=AO.add)
            # group reduce into [8, 2] col-pairs per channel tile (base 0)
            gstats = ps_sm.tile([GPT, 2 * NT], f32, tag="sm")
            for t in range(NT):
                nc.tensor.matmul(gstats[:, 2 * t:2 * t + 2], g16[:],
                                 stats[:, 2 * t:2 * t + 2], start=True, stop=True)
            # wk cols per tile t: 4t mu, 4t+1 rsig, 4t+2 ex2, 4t+3 scratch
            wk = sb2.tile([GPT, 4 * NT], f32, tag="wk")
            wk2 = sb2.tile([GPT, 2 * NT], f32, tag="wk2")
            for t in range(NT):
                nc.vector.tensor_scalar_mul(wk[:, 4 * t:4 * t + 1],
                                            gstats[:, 2 * t:2 * t + 1], 1.0 / GN_N)
                nc.vector.tensor_scalar_mul(wk[:, 4 * t + 2:4 * t + 3],
                                            gstats[:, 2 * t + 1:2 * t + 2],
                                            1.0 / GN_N)
                # mu^2 -> scratch; var = ex2 - mu^2 (+eps); rsig = 1/sqrt
                nc.vector.tensor_mul(wk[:, 4 * t + 3:4 * t + 4],
                                     wk[:, 4 * t:4 * t + 1], wk[:, 4 * t:4 * t + 1])
                nc.vector.tensor_sub(wk2[:, 2 * t:2 * t + 1],
                                     wk[:, 4 * t + 2:4 * t + 3],
                                     wk[:, 4 * t + 3:4 * t + 4])
                nc.vector.tensor_scalar_add(wk2[:, 2 * t:2 * t + 1],
                                            wk2[:, 2 * t:2 * t + 1], EPS)
                nc.scalar.sqrt(wk2[:, 2 * t + 1:2 * t + 2], wk2[:, 2 * t:2 * t + 1])
                nc.vector.reciprocal(wk[:, 4 * t + 1:4 * t + 2],
                                     wk2[:, 2 * t + 1:2 * t + 2])
            # scatter to channels + per-channel scale/bias
            # scb cols: 2t = sc (gamma*rsig), 2t+1 = tc (beta - mu*sc), 8+t tmp
            scb = sb2.tile([128, 12], f32, tag="scb")
            for t in range(NT):
                cst = ps_sm.tile([128, 2], f32, tag="sm")
                nc.tensor.matmul(cst[:], g16T[:], wk[:, 4 * t:4 * t + 2],
                                 start=True, stop=True)
                nc.vector.tensor_mul(scb[:, 2 * t:2 * t + 1], cst[:, 1:2],
                                     colv[:, t:t + 1])
                nc.vector.tensor_mul(scb[:, 8 + t:9 + t], cst[:, 0:1],
                                     scb[:, 2 * t:2 * t + 1])
                nc.vector.tensor_sub(scb[:, 2 * t + 1:2 * t + 2],
                                     colv[:, 4 + t:5 + t], scb[:, 8 + t:9 + t])

            # ---------- K^T and V projections from cond ----------
            cT = [sb2.tile([128, L], bf16, tag=f"cT{j}", name=f"cT{j}")
                  for j in range(KT)]
            for j in range(KT):
                nc.sync.dma_start(cT[j][:], condT_d[b, 128 * j:128 * (j + 1), :])
            kT = [sb2.tile([128, L], bf16, tag=f"kT{t}", name=f"kT{t}")
                  for t in range(NT)]
            v_sb = sb2.tile([L, C], bf16, tag="v_sb")
            for t in range(NT):
                cs = slice(128 * t, 128 * (t + 1))
                pk = ps_q.tile([128, CH], f32, tag="q")
                for j in range(KT):
                    nc.tensor.matmul(pk[:, 0:L], kwT[j][:, cs], cT[j][:],
                                     start=(j == 0), stop=(j == KT - 1))
                nc.scalar.activation(kT[t][:], pk[:, 0:L], AF.Identity,
                                     bias=kbs[:, t:t + 1], scale=scale_col[:])
                pv = ps_at.tile([128, CH], f32, tag="at")
                for j in range(KT):
                    nc.tensor.matmul(pv[0:L, 0:128], cT[j][:], vwT[j][:, cs],
                                     start=(j == 0), stop=(j == KT - 1))
                nc.vector.tensor_add(v_sb[:, cs], pv[0:L, 0:128], vb_bc[:, cs])

            # ---------- hw-chunk pipeline ----------
            for cix in range(NCH):
                cs = slice(CH * cix, CH * (cix + 1))
                # groupnorm apply (bf16 out)
                xn = [sb2.tile([128, CH], bf16, tag=f"xn{t}", name=f"xn{t}")
                      for t in range(NT)]
                for t in range(NT):
                    nc.scalar.activation(xn[t][:], xb[t][:, cs], AF.Identity,
                                         bias=scb[:, 2 * t + 1:2 * t + 2],
                                         scale=scb[:, 2 * t:2 * t + 1])
                # q projection (+bias fused into PSUM->SBUF cast)
                q_sb = [sb2.tile([128, CH], bf16, tag=f"q{m}", name=f"qsb{m}")
                        for m in range(NT)]
                for m in range(NT):
                    ms = slice(128 * m, 128 * (m + 1))
                    pq = ps_q.tile([128, CH], f32, tag="q")
                    for k in range(NT):
                        nc.tensor.matmul(pq[:], qwT[k][:, ms], xn[k][:],
                                         start=(k == 0), stop=(k == NT - 1))
                    nc.vector.tensor_scalar_add(q_sb[m][:], pq[:],
                                                colv[:, 8 + m:9 + m])
                # attention: per-head logits^T -> exp (bf16)
                eh = [sb2.tile([L, CH], bf16, tag=f"eh{h}", name=f"eh{h}")
                      for h in range(NH)]
                for h in range(NH):
                    t_, off = h // 2, 64 * (h % 2)
                    pqk = ps_at.tile([128, CH], f32, tag="at")
                    nc.tensor.matmul(pqk[0:L, :], kT[t_][off:off + 64, :],
                                     q_sb[t_][off:off + 64, :],
                                     start=True, stop=True)
                    nc.scalar.activation(eh[h][:], pqk[0:L, :], AF.Exp)
                # AV (pair-packed) + PE-replicated sums + normalize
                prj = [sb2.tile([128, CH], bf16, tag=f"pi{p}", name=f"pi{p}")
                       for p in range(NT)]
                for p in range(NT):
                    psm = ps_sm.tile([128, CH], f32, tag="sm")
                    pav = ps_at.tile([128, CH], f32, tag="at")
                    for h in (2 * p, 2 * p + 1):
                        off = 64 * (h % 2)
                        # sum of exp replicated over this head's 64 rows
                        nc.tensor.matmul(psm[off:off + 64, :], ones77[:],
                                         eh[h][:], start=True, stop=True)
                        nc.tensor.matmul(pav[off:off + 64, :],
                                         v_sb[:, 64 * h:64 * h + 64], eh[h][:],
                                         start=True, stop=True)
                    rcp = sb2.tile([128, CH], f32, tag=f"rcp{p % 2}",
                                   name=f"rcp{p}")
                    nc.vector.reciprocal(rcp[:], psm[:])
                    nc.vector.tensor_mul(prj[p][:], pav[:], rcp[:])
                # output projection + bias + residual
                for m in range(NT):
                    ms = slice(128 * m, 128 * (m + 1))
                    po = ps_o.tile([128, CH], f32, tag="o")
                    for k in range(NT):
                        nc.tensor.matmul(po[:], pwT[k][:, ms], prj[k][:],
                                         start=(k == 0), stop=False)
                    nc.tensor.matmul(po[:], pb_row[:, ms], ones_bf[:],
                                     start=False, stop=True)
                    xr = sb3.tile([128, CH], f32, tag="xr")
                    nc.sync.dma_start(xr[:], x_d[b, ms, cs])
                    osb = sb3.tile([128, CH], f32, tag="osb")
                    nc.vector.tensor_add(osb[:], po[:], xr[:])
                    nc.sync.dma_start(out_d[b, ms, cs], osb[:])

    nc.compile()
    return nc


_NC_CACHE = None


def _get_nc():
    global _NC_CACHE
    if _NC_CACHE is None:
        _NC_CACHE = _build_nc()
    return _NC_CACHE


def make_in_maps(x, cond, gamma, beta, q_w, q_b, k_w, k_b, v_w, v_b,
                 proj_w, proj_b, scale):
    x = np.asarray(x, np.float32).reshape(B, C, HW)
    condT = np.asarray(cond, np.float32).transpose(0, 2, 1).astype(BF16)
    qwT = np.ascontiguousarray(np.asarray(q_w, np.float32).T).astype(BF16)
    kwT = np.ascontiguousarray(np.asarray(k_w, np.float32).T).astype(BF16)
    vwT = np.ascontiguousarray(np.asarray(v_w, np.float32).T).astype(BF16)
    pwT = np.ascontiguousarray(np.asarray(proj_w, np.float32).T).astype(BF16)
    g16 = np.zeros((128, GPT), np.float32)
    for p in range(128):
        g16[p, p // GS] = 1
    g16T = np.ascontiguousarray(g16.T)
    com = dict(
        qwT=qwT, kwT=kwT, vwT=vwT, pwT=pwT,
        gamma=np.asarray(gamma, np.float32).reshape(C, 1),
        beta=np.asarray(beta, np.float32).reshape(C, 1),
        qb=np.asarray(q_b, np.float32).reshape(C, 1),
        kb=np.asarray(k_b, np.float32).reshape(C, 1),
        vb=np.asarray(v_b, np.float32).reshape(1, C),
        pb=np.asarray(proj_b, np.float32).reshape(1, C).astype(BF16),
        scale=np.asarray(scale, np.float32).reshape(1, 1),
        g16=g16, g16T=g16T,
    )
    in_maps = []
    for cix in range(N_CORES):
        bs = slice(B_LOC * cix, B_LOC * (cix + 1))
        m = dict(com)
        m["x"] = np.ascontiguousarray(x[bs])
        m["condT"] = np.ascontiguousarray(condT[bs])
        in_maps.append(m)
    return in_maps


def kernel(x, cond, gamma, beta, q_w, q_b, k_w, k_b, v_w, v_b,
           proj_w, proj_b, scale):
    nc = _get_nc()
    in_maps = make_in_maps(x, cond, gamma, beta, q_w, q_b, k_w, k_b,
                           v_w, v_b, proj_w, proj_b, scale)
    res = run_bass_kernel_spmd(nc, in_maps, core_ids=list(range(N_CORES)))
    out = np.concatenate([r["out"] for r in res.results], axis=0)
    return out.reshape(B, C, H, W).astype(np.float32)
